# revision 1
# baseline (speedup 1.0000x reference)
"""MultiHeadAttention TRN2 kernel.

Math (B=2, H=16, S=2048, D=128, F=256, DIM=2048), all fp32:
  Q = einsum('bhsf,hfd', q, Wq) + bq ; K likewise ; V = einsum('bhse,hed', v, Wv) + bv
  P = softmax(Q K^T / 16) ; o = P V ; out = concat_h(o) @ Wo + bo

Sharding: core c -> batch b=c//4, heads hg=(c%4)*4 .. +4 (tensor parallel over
heads). Each core computes its 4 heads' attention and the partial Wo product
(contraction over its 128*4=512 rows of Wo). Host sums the 4 partials per
batch and adds bo. No device collectives.

Device layout (per core, everything transposed on the host for free):
  qT  [4,2,128,2048] (head j, f-chunk, f, s)   kT same
  vT  [4,128,2048]   (j, e, s)
  wq/wk packed [128, 8*128] (f, (j,fc,d))      wv [128, 4*128] (e, (j,d))
  bq/bk [128,4] (d, j)   bv [128, 4*128] replicated over partitions
  wo [4,128,2048] (j, d, n)
  out_p [2048,2048] = partial (s, n)

All matmuls run as float32r (1 cyc/row at N>=256, full fp32 data).
"""

import os
import sys

import numpy as np

B, H, S, D, F = 2, 16, 2048, 128, 256
DIM = H * D
NC = 8
HPC = 4  # heads per core
SC512 = S // 512  # 4
NKT = S // 128  # 16

_BUILT = None
TRACE = False
LAST_RESULTS = None


def _import_concourse():
    try:
        import concourse.bass  # noqa: F401
    except ImportError:
        sys.path.insert(0, "/opt/trn_rl_repo")


def _build():
    _import_concourse()
    from contextlib import ExitStack

    import concourse.bass as bass
    import concourse.mybir as mybir
    import concourse.tile as tile

    f32 = mybir.dt.float32
    FR = mybir.dt.float32r
    AF = mybir.ActivationFunctionType

    nc = bass.Bass(target_bir_lowering=False)

    qT_d = nc.dram_tensor("qT", [HPC, 2, 128, S], FR, kind="ExternalInput")
    kT_d = nc.dram_tensor("kT", [HPC, 2, 128, S], FR, kind="ExternalInput")
    vT_d = nc.dram_tensor("vT", [HPC, 128, S], FR, kind="ExternalInput")
    wq_d = nc.dram_tensor("wq", [128, HPC * 2 * 128], FR, kind="ExternalInput")
    wk_d = nc.dram_tensor("wk", [128, HPC * 2 * 128], FR, kind="ExternalInput")
    wv_d = nc.dram_tensor("wv", [128, HPC * 128], FR, kind="ExternalInput")
    bq_d = nc.dram_tensor("bq", [128, HPC], f32, kind="ExternalInput")
    bk_d = nc.dram_tensor("bk", [128, HPC], f32, kind="ExternalInput")
    bv_d = nc.dram_tensor("bv", [128, HPC * 128], f32, kind="ExternalInput")
    wo_d = nc.dram_tensor("wo", [HPC, 128, DIM], FR, kind="ExternalInput")
    ones_d = nc.dram_tensor("ones", [128, 128], FR, kind="ExternalInput")
    out_d = nc.dram_tensor("out_p", [S, DIM], f32, kind="ExternalOutput")

    with ExitStack() as ctx:
        tc = ctx.enter_context(tile.TileContext(nc))
        consts = ctx.enter_context(tc.tile_pool(name="consts", bufs=1))
        raw = ctx.enter_context(tc.tile_pool(name="raw", bufs=5))
        big = ctx.enter_context(tc.tile_pool(name="big", bufs=2))
        otn_pool = ctx.enter_context(tc.tile_pool(name="otn", bufs=4))
        sm = ctx.enter_context(tc.tile_pool(name="sm", bufs=2))
        wop = ctx.enter_context(tc.tile_pool(name="wop", bufs=8))
        ps = ctx.enter_context(tc.tile_pool(name="ps", bufs=1, space="PSUM"))

        # ---- constants -------------------------------------------------
        ones_full = consts.tile([128, 128], FR)
        nc.sync.dma_start(out=ones_full, in_=ones_d[:])

        wq_sb = consts.tile([128, HPC * 2 * 128], FR)
        nc.scalar.dma_start(out=wq_sb, in_=wq_d[:])
        wk_sb = consts.tile([128, HPC * 2 * 128], FR)
        nc.scalar.dma_start(out=wk_sb, in_=wk_d[:])
        wv_sb = consts.tile([128, HPC * 128], FR)
        nc.scalar.dma_start(out=wv_sb, in_=wv_d[:])
        bq_sb = consts.tile([128, HPC], f32)
        nc.sync.dma_start(out=bq_sb, in_=bq_d[:])
        bk_sb = consts.tile([128, HPC], f32)
        nc.sync.dma_start(out=bk_sb, in_=bk_d[:])
        bv_sb = consts.tile([128, HPC * 128], f32)
        nc.sync.dma_start(out=bv_sb, in_=bv_d[:])

        wo_sb = {}

        # ---- P3 group emitter (interleaved into head-3 P2 + tail) ------
        store_q = [nc.gpsimd, nc.sync, nc.scalar]
        p3_state = {"n": 0}
        p3_pending = []

        def emit_p3_group(dc, sc, tail):
            csl = slice(sc * 128, (sc + 1) * 128)
            dsl = slice(dc * 512, (dc + 1) * 512)
            pw = ps.tile([128, 512], f32, tag="w", bufs=2, name=f"pw{dc}_{sc}")
            for j in range(HPC):
                nc.tensor.matmul(pw, otn[j][:, csl], wo_sb[dc, j],
                                 start=(j == 0), stop=(j == HPC - 1))
            ow = sm.tile([128, 512], f32, tag="ow", bufs=3, name=f"ow{dc}_{sc}")
            # during interleave keep drains off ACT (the bottleneck engine)
            if tail and p3_state["n"] % 2 == 0:
                nc.scalar.copy(out=ow, in_=pw)
            else:
                nc.vector.tensor_copy(out=ow, in_=pw)
            store_q[p3_state["n"] % 3].dma_start(out=out_d[csl, dsl], in_=ow)
            p3_state["n"] += 1

        otn = []

        # ---- P1: load + project head j (units interleaved into P2) -----
        def emit_p1_dmas(j):
            qa = raw.tile([128, S], FR, tag="raw", name=f"qa{j}")
            nc.sync.dma_start(out=qa, in_=qT_d[j, 0])
            va = raw.tile([128, S], FR, tag="raw", name=f"va{j}")
            nc.gpsimd.dma_start(out=va, in_=vT_d[j])
            ka = raw.tile([128, S], FR, tag="raw", name=f"ka{j}")
            nc.sync.dma_start(out=ka, in_=kT_d[j, 0])
            qb = raw.tile([128, S], FR, tag="raw", name=f"qb{j}")
            nc.gpsimd.dma_start(out=qb, in_=qT_d[j, 1])
            kb = raw.tile([128, S], FR, tag="raw", name=f"kb{j}")
            # head 0's kb rides the scalar queue ahead of the wo preload so
            # the ramp isn't gated on 3MB queued behind one engine
            (nc.scalar if j == 0 else nc.gpsimd).dma_start(out=kb, in_=kT_d[j, 1])
            QT = big.tile([128, S], FR, tag="QT", name=f"QT{j}")
            KT = big.tile([128, S], FR, tag="KT", name=f"KT{j}")
            Vsb = big.tile([128, S], FR, tag="V", name=f"V{j}")
            return qa, qb, ka, kb, va, QT, KT, Vsb

        def p1_units(j, qa, qb, ka, kb, va, QT, KT, Vsb):
            units = []
            for sc in range(SC512):
                ssl = slice(sc * 512, (sc + 1) * 512)

                def u_q(ssl=ssl, sc=sc):
                    pq = ps.tile([128, 512], f32, tag="w", bufs=2,
                                 name=f"pq{j}_{sc}")
                    nc.tensor.matmul(pq, wq_sb[:, (j * 2 + 0) * 128 : (j * 2 + 1) * 128],
                                     qa[:, ssl], start=True, stop=False)
                    nc.tensor.matmul(pq, wq_sb[:, (j * 2 + 1) * 128 : (j * 2 + 2) * 128],
                                     qb[:, ssl], start=False, stop=True)
                    nc.vector.tensor_scalar_add(out=QT[:, ssl], in0=pq,
                                                scalar1=bq_sb[:, j : j + 1])

                def u_k(ssl=ssl, sc=sc):
                    pk = ps.tile([128, 512], f32, tag="w", bufs=2,
                                 name=f"pk{j}_{sc}")
                    nc.tensor.matmul(pk, wk_sb[:, (j * 2 + 0) * 128 : (j * 2 + 1) * 128],
                                     ka[:, ssl], start=True, stop=False)
                    nc.tensor.matmul(pk, wk_sb[:, (j * 2 + 1) * 128 : (j * 2 + 2) * 128],
                                     kb[:, ssl], start=False, stop=True)
                    nc.vector.tensor_scalar_add(out=KT[:, ssl], in0=pk,
                                                scalar1=bk_sb[:, j : j + 1])

                units += [u_q, u_k]
            for kt0 in range(0, NKT, 2):

                def u_v(kt0=kt0):
                    for kt in (kt0, kt0 + 1):
                        csl = slice(kt * 128, (kt + 1) * 128)
                        pv = ps.tile([128, 512], f32, tag="w", bufs=2,
                                     name=f"pv{j}_{kt}")
                        nc.tensor.matmul(pv[:, 0:128], va[:, csl],
                                         wv_sb[:, j * 128 : (j + 1) * 128],
                                         start=True, stop=True)
                        nc.vector.tensor_add(out=Vsb[:, csl], in0=pv[:, 0:128],
                                             in1=bv_sb[:, j * 128 : (j + 1) * 128])

                units.append(u_v)
            return units

        hd = emit_p1_dmas(0)
        for dc in range(DIM // 512):
            for j in range(HPC):
                w = wop.tile([128, 512], FR, tag="wo", bufs=16,
                             name=f"wo{dc}_{j}")
                nc.scalar.dma_start(out=w, in_=wo_d[j, :, dc * 512 : (dc + 1) * 512])
                wo_sb[dc, j] = w
        for u in p1_units(0, *hd):
            u()
        p1_queue = []
        for j in range(HPC):
            _, _, _, _, _, QT, KT, Vsb = hd
            if j + 1 < HPC:
                hd = emit_p1_dmas(j + 1)
                p1_queue = p1_units(j + 1, *hd)

            # ---- P2: attention head j ----------------------------------
            oTn = otn_pool.tile([128, S], FR, tag="otn", name=f"oTn{j}")
            otn.append(oTn)
            for qc in range(SC512):
                qsl = slice(qc * 512, (qc + 1) * 512)
                po = ps.tile([128, 512], f32, tag="o", bufs=2, name=f"po{j}_{qc}")
                pr = ps.tile([128, 512], f32, tag="r", bufs=1, name=f"pr{j}_{qc}")

                def emit_pscore(kt):
                    csl = slice(kt * 128, (kt + 1) * 128)
                    t = ps.tile([128, 512], f32, tag="s", bufs=3,
                                name=f"ps{j}_{qc}_{kt}")
                    nc.tensor.matmul(t, KT[:, csl], QT[:, qsl],
                                     start=True, stop=True)
                    return t

                # software pipeline: pscore(kt+1) is emitted before po(kt)
                # so PE's in-order queue keeps ACT fed with score tiles
                # while po waits on exp(kt); otherwise every exp gets a
                # PE->ACT round-trip bubble on the bottleneck engine
                cur = emit_pscore(0)
                for kt in range(NKT):
                    csl = slice(kt * 128, (kt + 1) * 128)
                    pT = sm.tile([128, 512], FR, tag="pT", bufs=3, name=f"pT{j}_{qc}_{kt}")
                    nc.scalar.activation(out=pT, in_=cur, func=AF.Exp,
                                         bias=0.0, scale=0.0625)
                    if kt + 1 < NKT:
                        cur = emit_pscore(kt + 1)
                    nc.tensor.matmul(po, Vsb[:, csl], pT,
                                     start=(kt == 0), stop=(kt == NKT - 1))
                    nc.tensor.matmul(pr, ones_full, pT,
                                     start=(kt == 0), stop=(kt == NKT - 1))
                    # PE slack under the ACT exp bottleneck: fold one output
                    # projection group per kt slot once its tokens are done
                    if p3_pending:
                        emit_p3_group(*p3_pending.pop(0), tail=False)
                    elif p1_queue and kt % 2 == 0:
                        p1_queue.pop(0)()
                rr = sm.tile([128, 512], f32, tag="rr_sb", bufs=2, name=f"rr{j}_{qc}")
                nc.vector.reciprocal(out=rr, in_=pr)
                nc.vector.tensor_mul(out=oTn[:, qsl], in0=po, in1=rr)
                if j == HPC - 1:
                    p3_pending.extend(
                        (dc, sc)
                        for sc in range(qc * 4, (qc + 1) * 4)
                        for dc in range(DIM // 512))
            for u in p1_queue:
                u()
            p1_queue = []

        # ---- P3 tail: groups not hidden inside P2 ----------------------
        while p3_pending:
            emit_p3_group(*p3_pending.pop(0), tail=True)

    _split_excess_waits(nc)
    return nc


def _split_excess_waits(nc):
    """Compute-engine instructions (Matmult, TensorScalarPtr, ...) only have
    one sync-wait slot in walrus codegen. Split any excess waits onto
    same-engine NoOps inserted just before the instruction."""
    import concourse.mybir as mybir

    n = 0
    for func in nc.m.functions:
        for block in func.blocks:
            out = []
            for inst in block.instructions:
                si = getattr(inst, "sync_info", None)
                if si is not None and si.on_wait and len(si.on_wait) > 1:
                    for w in si.on_wait[:-1]:
                        nop = mybir.InstNoOp(
                            name=f"wsplit_{n}",
                            engine=inst.engine,
                            sync_info=mybir.SyncInfo(on_wait=[w], on_update=[]),
                            bass_nofuse=True,
                        )
                        n += 1
                        out.append(nop)
                    inst.sync_info = mybir.SyncInfo(
                        on_wait=[si.on_wait[-1]], on_update=si.on_update)
                out.append(inst)
            block.instructions[:] = out
    return n


def _prep_core(c, q, k, v, Wq, Wk, Wv, bq, bk, bv, Wo):
    b = c // 4
    hs = slice((c % 4) * HPC, (c % 4) * HPC + HPC)
    qT = np.ascontiguousarray(q[b, hs].transpose(0, 2, 1)).reshape(HPC, 2, 128, S)
    kT = np.ascontiguousarray(k[b, hs].transpose(0, 2, 1)).reshape(HPC, 2, 128, S)
    vT = np.ascontiguousarray(v[b, hs].transpose(0, 2, 1))
    wq = np.ascontiguousarray(
        Wq[hs].reshape(HPC, 2, 128, D).transpose(2, 0, 1, 3)).reshape(128, HPC * 2 * 128)
    wk = np.ascontiguousarray(
        Wk[hs].reshape(HPC, 2, 128, D).transpose(2, 0, 1, 3)).reshape(128, HPC * 2 * 128)
    wv = np.ascontiguousarray(Wv[hs].transpose(1, 0, 2)).reshape(128, HPC * 128)
    bqT = np.ascontiguousarray(bq[hs].T)
    bkT = np.ascontiguousarray(bk[hs].T)
    bvr = np.ascontiguousarray(
        np.broadcast_to(bv[hs][:, None, :], (HPC, 128, D)).transpose(1, 0, 2)
    ).reshape(128, HPC * D)
    wo = np.ascontiguousarray(Wo.reshape(H, D, DIM)[hs])
    return {
        "qT": qT, "kT": kT, "vT": vT, "wq": wq, "wk": wk, "wv": wv,
        "bq": bqT, "bk": bkT, "bv": bvr, "wo": wo,
        "ones": np.ones((128, 128), dtype=np.float32),
    }


def kernel(q, k, v, Wq, Wk, Wv, bq, bk, bv, Wo, bo):
    global _BUILT, LAST_RESULTS
    _import_concourse()
    from concourse.bass_utils import run_bass_kernel_spmd

    args = [np.asarray(x, dtype=np.float32)
            for x in (q, k, v, Wq, Wk, Wv, bq, bk, bv, Wo)]
    if _BUILT is None:
        _BUILT = _build()
    in_maps = [_prep_core(c, *args) for c in range(NC)]
    res = run_bass_kernel_spmd(_BUILT, in_maps, core_ids=list(range(NC)),
                               trace=TRACE)
    LAST_RESULTS = res
    bo = np.asarray(bo, dtype=np.float32)
    outs = [res.results[c]["out_p"] for c in range(NC)]
    out = np.stack([
        outs[0] + outs[1] + outs[2] + outs[3] + bo,
        outs[4] + outs[5] + outs[6] + outs[7] + bo,
    ]).astype(np.float32)
    return out



# revision 2
# speedup vs baseline: 4.4434x; 4.4434x over previous
"""MultiHeadAttention TRN2 kernel.

Math (B=2, H=16, S=2048, D=128, F=256, DIM=2048):
  Q = einsum('bhsf,hfd', q, Wq) + bq ; K likewise ; V = einsum('bhse,hed', v, Wv) + bv
  P = softmax(Q K^T / 16) ; o = P V ; out = concat_h(o) @ Wo + bo

Sharding: core c -> batch b=c//4, heads hg=(c%4)*4 .. +4 (tensor parallel over
heads). Each core computes its 4 heads' attention and the partial Wo product
(contraction over its 128*4=512 rows of Wo). A ReduceScatter(add) over each
batch's 4-core group sums the partials on device; core c returns rows
512*(c%4) .. +512 of its batch's output. Host concatenates and adds bo.

The end-to-end metric here is the warm kernel() wall time, which is dominated
by host<->device transfer over the axon tunnel (~31 MB/s up, ~7 MB/s down),
not device compute. So everything on the wire is fp16 (inputs ~103 MB, output
16 MB); matmuls run fp16 with fp32 PSUM accumulation (rel err ~2e-3, gate is
2e-2).

Device layout (per core, packed on the host):
  qT  [4,2,128,2048] (head j, f-chunk, f, s)   kT same
  vT  [4,128,2048]   (j, e, s)
  wq/wk packed [128, 8*128] (f, (j,fc,d))      wv [128, 4*128] (e, (j,d))
  bq/bk [128,4] (d, j)   bv [128, 4*128] replicated over partitions (fp32)
  wo [4,128,2048] (j, d, n)
  out_s [512,2048] fp16 = this core's S-rows slice of its batch's output

All matmuls run fp16 (stationary+moving) into fp32 PSUM.
"""

import os
import sys

import numpy as np

B, H, S, D, F = 2, 16, 2048, 128, 256
DIM = H * D
NC = 8
HPC = 4  # heads per core
SC512 = S // 512  # 4
NKT = S // 128  # 16
SOUT = S // 4  # 512 rows returned per core after ReduceScatter

_BUILT = None
TRACE = False
LAST_RESULTS = None


def _import_concourse():
    try:
        import concourse.bass  # noqa: F401
    except ImportError:
        sys.path.insert(0, "/opt/trn_rl_repo")


def _build():
    _import_concourse()
    from contextlib import ExitStack

    import concourse.bass as bass
    import concourse.mybir as mybir
    import concourse.tile as tile

    f32 = mybir.dt.float32
    f16 = mybir.dt.float16
    AF = mybir.ActivationFunctionType

    nc = bass.Bass(target_bir_lowering=False, num_devices=NC)

    qT_d = nc.dram_tensor("qT", [HPC, 2, 128, S], f16, kind="ExternalInput")
    kT_d = nc.dram_tensor("kT", [HPC, 2, 128, S], f16, kind="ExternalInput")
    vT_d = nc.dram_tensor("vT", [HPC, 128, S], f16, kind="ExternalInput")
    wq_d = nc.dram_tensor("wq", [128, HPC * 2 * 128], f16, kind="ExternalInput")
    wk_d = nc.dram_tensor("wk", [128, HPC * 2 * 128], f16, kind="ExternalInput")
    wv_d = nc.dram_tensor("wv", [128, HPC * 128], f16, kind="ExternalInput")
    bq_d = nc.dram_tensor("bq", [128, HPC], f32, kind="ExternalInput")
    bk_d = nc.dram_tensor("bk", [128, HPC], f32, kind="ExternalInput")
    bv_d = nc.dram_tensor("bv", [128, HPC * 128], f32, kind="ExternalInput")
    wo_d = nc.dram_tensor("wo", [HPC, 128, DIM], f16, kind="ExternalInput")
    ones_d = nc.dram_tensor("ones", [128, 128], f16, kind="ExternalInput")
    out_d = nc.dram_tensor("out_s", [SOUT, DIM], f16, kind="ExternalOutput")

    with ExitStack() as ctx:
        tc = ctx.enter_context(tile.TileContext(nc))
        consts = ctx.enter_context(tc.tile_pool(name="consts", bufs=1))
        raw = ctx.enter_context(tc.tile_pool(name="raw", bufs=5))
        big = ctx.enter_context(tc.tile_pool(name="big", bufs=2))
        otn_pool = ctx.enter_context(tc.tile_pool(name="otn", bufs=4))
        sm = ctx.enter_context(tc.tile_pool(name="sm", bufs=2))
        wop = ctx.enter_context(tc.tile_pool(name="wop", bufs=8))
        ps = ctx.enter_context(tc.tile_pool(name="ps", bufs=1, space="PSUM"))
        dram = ctx.enter_context(tc.tile_pool(name="dram", bufs=1, space="DRAM"))

        # partial output (pre-reduce) and the ReduceScatter result, on-device
        out_pre = dram.tile([S, DIM], f16)
        out_rs = dram.tile([SOUT, DIM], f16)

        # ---- constants -------------------------------------------------
        ones_full = consts.tile([128, 128], f16)
        nc.sync.dma_start(out=ones_full, in_=ones_d[:])

        wq_sb = consts.tile([128, HPC * 2 * 128], f16)
        nc.scalar.dma_start(out=wq_sb, in_=wq_d[:])
        wk_sb = consts.tile([128, HPC * 2 * 128], f16)
        nc.scalar.dma_start(out=wk_sb, in_=wk_d[:])
        wv_sb = consts.tile([128, HPC * 128], f16)
        nc.scalar.dma_start(out=wv_sb, in_=wv_d[:])
        bq_sb = consts.tile([128, HPC], f32)
        nc.sync.dma_start(out=bq_sb, in_=bq_d[:])
        bk_sb = consts.tile([128, HPC], f32)
        nc.sync.dma_start(out=bk_sb, in_=bk_d[:])
        bv_sb = consts.tile([128, HPC * 128], f32)
        nc.sync.dma_start(out=bv_sb, in_=bv_d[:])

        wo_sb = {}

        # ---- P3 group emitter (interleaved into head-3 P2 + tail) ------
        store_q = [nc.gpsimd, nc.sync, nc.scalar]
        p3_state = {"n": 0}
        p3_pending = []

        def emit_p3_group(dc, sc, tail):
            csl = slice(sc * 128, (sc + 1) * 128)
            dsl = slice(dc * 512, (dc + 1) * 512)
            pw = ps.tile([128, 512], f32, tag="w", bufs=2, name=f"pw{dc}_{sc}")
            for j in range(HPC):
                nc.tensor.matmul(pw, otn[j][:, csl], wo_sb[dc, j],
                                 start=(j == 0), stop=(j == HPC - 1))
            ow = sm.tile([128, 512], f16, tag="ow", bufs=3, name=f"ow{dc}_{sc}")
            # during interleave keep drains off ACT (the bottleneck engine)
            if tail and p3_state["n"] % 2 == 0:
                nc.scalar.copy(out=ow, in_=pw)
            else:
                nc.vector.tensor_copy(out=ow, in_=pw)
            store_q[p3_state["n"] % 3].dma_start(out=out_pre[csl, dsl], in_=ow)
            p3_state["n"] += 1

        otn = []

        # ---- P1: load + project head j (units interleaved into P2) -----
        def emit_p1_dmas(j):
            qa = raw.tile([128, S], f16, tag="raw", name=f"qa{j}")
            nc.sync.dma_start(out=qa, in_=qT_d[j, 0])
            va = raw.tile([128, S], f16, tag="raw", name=f"va{j}")
            nc.gpsimd.dma_start(out=va, in_=vT_d[j])
            ka = raw.tile([128, S], f16, tag="raw", name=f"ka{j}")
            nc.sync.dma_start(out=ka, in_=kT_d[j, 0])
            qb = raw.tile([128, S], f16, tag="raw", name=f"qb{j}")
            nc.gpsimd.dma_start(out=qb, in_=qT_d[j, 1])
            kb = raw.tile([128, S], f16, tag="raw", name=f"kb{j}")
            # head 0's kb rides the scalar queue ahead of the wo preload so
            # the ramp isn't gated on queued MBs behind one engine
            (nc.scalar if j == 0 else nc.gpsimd).dma_start(out=kb, in_=kT_d[j, 1])
            QT = big.tile([128, S], f16, tag="QT", name=f"QT{j}")
            KT = big.tile([128, S], f16, tag="KT", name=f"KT{j}")
            Vsb = big.tile([128, S], f16, tag="V", name=f"V{j}")
            return qa, qb, ka, kb, va, QT, KT, Vsb

        def p1_units(j, qa, qb, ka, kb, va, QT, KT, Vsb):
            units = []
            for sc in range(SC512):
                ssl = slice(sc * 512, (sc + 1) * 512)

                def u_q(ssl=ssl, sc=sc):
                    pq = ps.tile([128, 512], f32, tag="w", bufs=2,
                                 name=f"pq{j}_{sc}")
                    nc.tensor.matmul(pq, wq_sb[:, (j * 2 + 0) * 128 : (j * 2 + 1) * 128],
                                     qa[:, ssl], start=True, stop=False)
                    nc.tensor.matmul(pq, wq_sb[:, (j * 2 + 1) * 128 : (j * 2 + 2) * 128],
                                     qb[:, ssl], start=False, stop=True)
                    nc.vector.tensor_scalar_add(out=QT[:, ssl], in0=pq,
                                                scalar1=bq_sb[:, j : j + 1])

                def u_k(ssl=ssl, sc=sc):
                    pk = ps.tile([128, 512], f32, tag="w", bufs=2,
                                 name=f"pk{j}_{sc}")
                    nc.tensor.matmul(pk, wk_sb[:, (j * 2 + 0) * 128 : (j * 2 + 1) * 128],
                                     ka[:, ssl], start=True, stop=False)
                    nc.tensor.matmul(pk, wk_sb[:, (j * 2 + 1) * 128 : (j * 2 + 2) * 128],
                                     kb[:, ssl], start=False, stop=True)
                    nc.vector.tensor_scalar_add(out=KT[:, ssl], in0=pk,
                                                scalar1=bk_sb[:, j : j + 1])

                units += [u_q, u_k]
            for kt0 in range(0, NKT, 2):

                def u_v(kt0=kt0):
                    for kt in (kt0, kt0 + 1):
                        csl = slice(kt * 128, (kt + 1) * 128)
                        pv = ps.tile([128, 512], f32, tag="w", bufs=2,
                                     name=f"pv{j}_{kt}")
                        nc.tensor.matmul(pv[:, 0:128], va[:, csl],
                                         wv_sb[:, j * 128 : (j + 1) * 128],
                                         start=True, stop=True)
                        nc.vector.tensor_add(out=Vsb[:, csl], in0=pv[:, 0:128],
                                             in1=bv_sb[:, j * 128 : (j + 1) * 128])

                units.append(u_v)
            return units

        hd = emit_p1_dmas(0)
        for dc in range(DIM // 512):
            for j in range(HPC):
                w = wop.tile([128, 512], f16, tag="wo", bufs=16,
                             name=f"wo{dc}_{j}")
                nc.scalar.dma_start(out=w, in_=wo_d[j, :, dc * 512 : (dc + 1) * 512])
                wo_sb[dc, j] = w
        for u in p1_units(0, *hd):
            u()
        p1_queue = []
        for j in range(HPC):
            _, _, _, _, _, QT, KT, Vsb = hd
            if j + 1 < HPC:
                hd = emit_p1_dmas(j + 1)
                p1_queue = p1_units(j + 1, *hd)

            # ---- P2: attention head j ----------------------------------
            oTn = otn_pool.tile([128, S], f16, tag="otn", name=f"oTn{j}")
            otn.append(oTn)
            for qc in range(SC512):
                qsl = slice(qc * 512, (qc + 1) * 512)
                po = ps.tile([128, 512], f32, tag="o", bufs=2, name=f"po{j}_{qc}")
                pr = ps.tile([128, 512], f32, tag="r", bufs=1, name=f"pr{j}_{qc}")

                def emit_pscore(kt):
                    csl = slice(kt * 128, (kt + 1) * 128)
                    t = ps.tile([128, 512], f32, tag="s", bufs=3,
                                name=f"ps{j}_{qc}_{kt}")
                    nc.tensor.matmul(t, KT[:, csl], QT[:, qsl],
                                     start=True, stop=True)
                    return t

                # software pipeline: pscore(kt+1) is emitted before po(kt)
                # so PE's in-order queue keeps ACT fed with score tiles
                # while po waits on exp(kt); otherwise every exp gets a
                # PE->ACT round-trip bubble on the bottleneck engine
                cur = emit_pscore(0)
                for kt in range(NKT):
                    csl = slice(kt * 128, (kt + 1) * 128)
                    pT = sm.tile([128, 512], f16, tag="pT", bufs=3, name=f"pT{j}_{qc}_{kt}")
                    nc.scalar.activation(out=pT, in_=cur, func=AF.Exp,
                                         bias=0.0, scale=0.0625)
                    if kt + 1 < NKT:
                        cur = emit_pscore(kt + 1)
                    nc.tensor.matmul(po, Vsb[:, csl], pT,
                                     start=(kt == 0), stop=(kt == NKT - 1))
                    nc.tensor.matmul(pr, ones_full, pT,
                                     start=(kt == 0), stop=(kt == NKT - 1))
                    # PE slack under the ACT exp bottleneck: fold one output
                    # projection group per kt slot once its tokens are done
                    if p3_pending:
                        emit_p3_group(*p3_pending.pop(0), tail=False)
                    elif p1_queue and kt % 2 == 0:
                        p1_queue.pop(0)()
                rr = sm.tile([128, 512], f32, tag="rr_sb", bufs=2, name=f"rr{j}_{qc}")
                nc.vector.reciprocal(out=rr, in_=pr)
                nc.vector.tensor_mul(out=oTn[:, qsl], in0=po, in1=rr)
                if j == HPC - 1:
                    p3_pending.extend(
                        (dc, sc)
                        for sc in range(qc * 4, (qc + 1) * 4)
                        for dc in range(DIM // 512))
            for u in p1_queue:
                u()
            p1_queue = []

        # ---- P3 tail: groups not hidden inside P2 ----------------------
        while p3_pending:
            emit_p3_group(*p3_pending.pop(0), tail=True)

        # ---- P4: sum the 4 per-core partials of this batch on device ---
        # ReduceScatter over the batch group: rank r keeps the r-th quarter
        # of the flattened [S, DIM] buffer = rows 512r..512(r+1).
        nc.gpsimd.collective_compute(
            "ReduceScatter",
            mybir.AluOpType.add,
            replica_groups=[[0, 1, 2, 3], [4, 5, 6, 7]],
            ins=[out_pre[:].opt()],
            outs=[out_rs[:].opt()],
        )
        nc.sync.dma_start(out=out_d[:], in_=out_rs[:])

    _split_excess_waits(nc)
    return nc


def _split_excess_waits(nc):
    """Compute-engine instructions (Matmult, TensorScalarPtr, ...) only have
    one sync-wait slot in walrus codegen. Split any excess waits onto
    same-engine NoOps inserted just before the instruction."""
    import concourse.mybir as mybir

    n = 0
    for func in nc.m.functions:
        for block in func.blocks:
            out = []
            for inst in block.instructions:
                si = getattr(inst, "sync_info", None)
                if si is not None and si.on_wait and len(si.on_wait) > 1:
                    for w in si.on_wait[:-1]:
                        nop = mybir.InstNoOp(
                            name=f"wsplit_{n}",
                            engine=inst.engine,
                            sync_info=mybir.SyncInfo(on_wait=[w], on_update=[]),
                            bass_nofuse=True,
                        )
                        n += 1
                        out.append(nop)
                    inst.sync_info = mybir.SyncInfo(
                        on_wait=[si.on_wait[-1]], on_update=si.on_update)
                out.append(inst)
            block.instructions[:] = out
    return n


def _prep_core(c, q16, k16, v16, Wq16, Wk16, Wv16, bq, bk, bv, Wo16):
    b = c // 4
    hs = slice((c % 4) * HPC, (c % 4) * HPC + HPC)
    f16 = np.float16
    qT = np.ascontiguousarray(q16[b, hs].transpose(0, 2, 1)).reshape(HPC, 2, 128, S)
    kT = np.ascontiguousarray(k16[b, hs].transpose(0, 2, 1)).reshape(HPC, 2, 128, S)
    vT = np.ascontiguousarray(v16[b, hs].transpose(0, 2, 1))
    wq = np.ascontiguousarray(
        Wq16[hs].reshape(HPC, 2, 128, D).transpose(2, 0, 1, 3)).reshape(128, HPC * 2 * 128)
    wk = np.ascontiguousarray(
        Wk16[hs].reshape(HPC, 2, 128, D).transpose(2, 0, 1, 3)).reshape(128, HPC * 2 * 128)
    wv = np.ascontiguousarray(Wv16[hs].transpose(1, 0, 2)).reshape(128, HPC * 128)
    bqT = np.ascontiguousarray(bq[hs].T)
    bkT = np.ascontiguousarray(bk[hs].T)
    bvr = np.ascontiguousarray(
        np.broadcast_to(bv[hs][:, None, :], (HPC, 128, D)).transpose(1, 0, 2)
    ).reshape(128, HPC * D)
    wo = np.ascontiguousarray(Wo16.reshape(H, D, DIM)[hs])
    return {
        "qT": qT, "kT": kT, "vT": vT, "wq": wq, "wk": wk, "wv": wv,
        "bq": bqT, "bk": bkT, "bv": bvr, "wo": wo,
        "ones": np.ones((128, 128), dtype=f16),
    }


def kernel(q, k, v, Wq, Wk, Wv, bq, bk, bv, Wo, bo):
    global _BUILT, LAST_RESULTS
    _import_concourse()
    from concourse.bass_utils import run_bass_kernel_spmd

    f16 = np.float16
    q16 = np.asarray(q, dtype=f16)
    k16 = np.asarray(k, dtype=f16)
    v16 = np.asarray(v, dtype=f16)
    Wq16 = np.asarray(Wq, dtype=f16)
    Wk16 = np.asarray(Wk, dtype=f16)
    Wv16 = np.asarray(Wv, dtype=f16)
    Wo16 = np.asarray(Wo, dtype=f16)
    bq = np.asarray(bq, dtype=np.float32)
    bk = np.asarray(bk, dtype=np.float32)
    bv = np.asarray(bv, dtype=np.float32)
    if _BUILT is None:
        _BUILT = _build()
    in_maps = [_prep_core(c, q16, k16, v16, Wq16, Wk16, Wv16, bq, bk, bv, Wo16)
               for c in range(NC)]
    res = run_bass_kernel_spmd(_BUILT, in_maps, core_ids=list(range(NC)),
                               trace=TRACE)
    LAST_RESULTS = res
    bo = np.asarray(bo, dtype=np.float32)
    out = np.stack([
        np.concatenate([res.results[c]["out_s"] for c in range(4)],
                       axis=0).astype(np.float32) + bo,
        np.concatenate([res.results[c]["out_s"] for c in range(4, 8)],
                       axis=0).astype(np.float32) + bo,
    ])
    return out


# revision 3
# speedup vs baseline: 11.9484x; 2.6890x over previous
"""MultiHeadAttention TRN2 kernel.

Math (B=2, H=16, S=2048, D=128, F=256, DIM=2048):
  Q = einsum('bhsf,hfd', q, Wq) + bq ; K likewise ; V = einsum('bhse,hed', v, Wv) + bv
  P = softmax(Q K^T / 16) ; o = P V ; out = concat_h(o) @ Wo + bo

The end-to-end metric is the warm kernel() wall time, dominated by
host<->device transfer over the axon tunnel (~20-40 MB/s), not device
compute (~0.5 ms/core). So the kernel minimizes wire bytes:
  - Q/K/V projections run on host in fp32 BLAS (cheap: ~11 GFLOP) and the
    projected tensors ship as fp16 (48 MB instead of 160+ MB of raw q/k/v
    plus weights).
  - Each core ships only half of its 4 heads' Wo rows; a 2-rank AllGather
    between batch-pair cores (c, c+4), which need identical rows,
    reconstructs the full set on device (8 MB instead of 16 MB).
  - The attention + output projection partials are summed across each
    batch's 4-core group with an on-device ReduceScatter; each core returns
    a [512, 2048] fp16 slice of the final output (16 MB down instead of
    128 MB of fp32 partials).

Sharding: core c -> batch b=c//4, heads hg=(c%4)*4 .. +4. Each core runs
attention for its 4 heads and the partial Wo product (contraction over its
4*128 rows of Wo). ReduceScatter(add) over [[0..3],[4..7]] leaves core c
with rows 512*(c%4) .. +512 of its batch's output. Host concatenates the
slices and adds bo.

Device layout (per core, packed on the host):
  QT  [4,128,2048] (head j, d, s) = (q Wq + bq)^T        KT same
  VT  [4,128,2048] (head j, token%128, (token//128, d)) block-transposed so
      VT[j][:, 128*kt:...] is [token, d] for token-chunk kt
  wo_half [2,128,2048] (j, d, n): heads 0-1 of the group on cores 0-3,
      heads 2-3 on cores 4-7
  out_s [512,2048] fp16

All matmuls run fp16 (stationary+moving) into fp32 PSUM.
"""

import os
import sys

import numpy as np

B, H, S, D, F = 2, 16, 2048, 128, 256
DIM = H * D
NC = 8
HPC = 4  # heads per core
SC512 = S // 512  # 4
NKT = S // 128  # 16
SOUT = S // 4  # 512 rows returned per core after ReduceScatter

_BUILT = None
TRACE = False
LAST_RESULTS = None


def _import_concourse():
    try:
        import concourse.bass  # noqa: F401
    except ImportError:
        sys.path.insert(0, "/opt/trn_rl_repo")


def _build():
    _import_concourse()
    from contextlib import ExitStack

    import concourse.bass as bass
    import concourse.mybir as mybir
    import concourse.tile as tile

    f32 = mybir.dt.float32
    f16 = mybir.dt.float16
    AF = mybir.ActivationFunctionType

    nc = bass.Bass(target_bir_lowering=False, num_devices=NC)

    qT_d = nc.dram_tensor("QT", [HPC, 128, S], f16, kind="ExternalInput")
    kT_d = nc.dram_tensor("KT", [HPC, 128, S], f16, kind="ExternalInput")
    vT_d = nc.dram_tensor("VT", [HPC, 128, S], f16, kind="ExternalInput")
    wo_d = nc.dram_tensor("wo_half", [2, 128, DIM], f16, kind="ExternalInput")
    out_d = nc.dram_tensor("out_s", [SOUT, DIM], f16, kind="ExternalOutput")

    with ExitStack() as ctx:
        tc = ctx.enter_context(tile.TileContext(nc))
        consts = ctx.enter_context(tc.tile_pool(name="consts", bufs=1))
        big = ctx.enter_context(tc.tile_pool(name="big", bufs=2))
        otn_pool = ctx.enter_context(tc.tile_pool(name="otn", bufs=4))
        sm = ctx.enter_context(tc.tile_pool(name="sm", bufs=2))
        wop = ctx.enter_context(tc.tile_pool(name="wop", bufs=8))
        ps = ctx.enter_context(tc.tile_pool(name="ps", bufs=1, space="PSUM"))
        dram = ctx.enter_context(tc.tile_pool(name="dram", bufs=1, space="DRAM"))

        wo_in = dram.tile([2, 128, DIM], f16)
        wo_full = dram.tile([HPC, 128, DIM], f16)
        out_pre = dram.tile([S, DIM], f16)
        out_rs = dram.tile([SOUT, DIM], f16)

        # ---- constants -------------------------------------------------
        ones_full = consts.tile([128, 128], f16)
        nc.vector.memset(ones_full[:], 1.0)

        # wo rows are shared between batch-pair cores (c, c+4): each ships
        # half, a 2-rank AllGather rebuilds the full [4,128,DIM] on device
        nc.scalar.dma_start(out=wo_in[:], in_=wo_d[:])
        nc.gpsimd.collective_compute(
            "AllGather",
            mybir.AluOpType.bypass,
            replica_groups=[[0, 4], [1, 5], [2, 6], [3, 7]],
            ins=[wo_in[:].opt()],
            outs=[wo_full[:].opt()],
        )

        wo_sb = {}
        for dc in range(DIM // 512):
            for j in range(HPC):
                w = wop.tile([128, 512], f16, tag="wo", bufs=16,
                             name=f"wo{dc}_{j}")
                nc.scalar.dma_start(out=w, in_=wo_full[j, :, dc * 512 : (dc + 1) * 512])
                wo_sb[dc, j] = w

        # ---- P3 group emitter (interleaved into head-3 P2 + tail) ------
        store_q = [nc.gpsimd, nc.sync, nc.scalar]
        p3_state = {"n": 0}
        p3_pending = []

        def emit_p3_group(dc, sc, tail):
            csl = slice(sc * 128, (sc + 1) * 128)
            dsl = slice(dc * 512, (dc + 1) * 512)
            pw = ps.tile([128, 512], f32, tag="w", bufs=2, name=f"pw{dc}_{sc}")
            for j in range(HPC):
                nc.tensor.matmul(pw, otn[j][:, csl], wo_sb[dc, j],
                                 start=(j == 0), stop=(j == HPC - 1))
            ow = sm.tile([128, 512], f16, tag="ow", bufs=3, name=f"ow{dc}_{sc}")
            # during interleave keep drains off ACT (the bottleneck engine)
            if tail and p3_state["n"] % 2 == 0:
                nc.scalar.copy(out=ow, in_=pw)
            else:
                nc.vector.tensor_copy(out=ow, in_=pw)
            store_q[p3_state["n"] % 3].dma_start(out=out_pre[csl, dsl], in_=ow)
            p3_state["n"] += 1

        otn = []

        # ---- P1: load head j's projected tensors -----------------------
        def emit_head_dmas(j):
            QT = big.tile([128, S], f16, tag="QT", name=f"QT{j}")
            nc.sync.dma_start(out=QT, in_=qT_d[j])
            KT = big.tile([128, S], f16, tag="KT", name=f"KT{j}")
            nc.gpsimd.dma_start(out=KT, in_=kT_d[j])
            Vsb = big.tile([128, S], f16, tag="V", name=f"V{j}")
            (nc.scalar if j == 0 else nc.sync).dma_start(out=Vsb, in_=vT_d[j])
            return QT, KT, Vsb

        hd = emit_head_dmas(0)
        for j in range(HPC):
            QT, KT, Vsb = hd
            if j + 1 < HPC:
                hd = emit_head_dmas(j + 1)

            # ---- P2: attention head j ----------------------------------
            oTn = otn_pool.tile([128, S], f16, tag="otn", name=f"oTn{j}")
            otn.append(oTn)
            for qc in range(SC512):
                qsl = slice(qc * 512, (qc + 1) * 512)
                po = ps.tile([128, 512], f32, tag="o", bufs=2, name=f"po{j}_{qc}")
                pr = ps.tile([128, 512], f32, tag="r", bufs=1, name=f"pr{j}_{qc}")

                def emit_pscore(kt):
                    csl = slice(kt * 128, (kt + 1) * 128)
                    t = ps.tile([128, 512], f32, tag="s", bufs=3,
                                name=f"ps{j}_{qc}_{kt}")
                    nc.tensor.matmul(t, KT[:, csl], QT[:, qsl],
                                     start=True, stop=True)
                    return t

                # software pipeline: pscore(kt+1) is emitted before po(kt)
                # so PE's in-order queue keeps ACT fed with score tiles
                # while po waits on exp(kt); otherwise every exp gets a
                # PE->ACT round-trip bubble on the bottleneck engine
                cur = emit_pscore(0)
                for kt in range(NKT):
                    csl = slice(kt * 128, (kt + 1) * 128)
                    pT = sm.tile([128, 512], f16, tag="pT", bufs=3, name=f"pT{j}_{qc}_{kt}")
                    nc.scalar.activation(out=pT, in_=cur, func=AF.Exp,
                                         bias=0.0, scale=0.0625)
                    if kt + 1 < NKT:
                        cur = emit_pscore(kt + 1)
                    nc.tensor.matmul(po, Vsb[:, csl], pT,
                                     start=(kt == 0), stop=(kt == NKT - 1))
                    nc.tensor.matmul(pr, ones_full, pT,
                                     start=(kt == 0), stop=(kt == NKT - 1))
                    # PE slack under the ACT exp bottleneck: fold one output
                    # projection group per kt slot once its tokens are done
                    if p3_pending:
                        emit_p3_group(*p3_pending.pop(0), tail=False)
                rr = sm.tile([128, 512], f32, tag="rr_sb", bufs=2, name=f"rr{j}_{qc}")
                nc.vector.reciprocal(out=rr, in_=pr)
                nc.vector.tensor_mul(out=oTn[:, qsl], in0=po, in1=rr)
                if j == HPC - 1:
                    p3_pending.extend(
                        (dc, sc)
                        for sc in range(qc * 4, (qc + 1) * 4)
                        for dc in range(DIM // 512))

        # ---- P3 tail: groups not hidden inside P2 ----------------------
        while p3_pending:
            emit_p3_group(*p3_pending.pop(0), tail=True)

        # ---- P4: sum the 4 per-core partials of this batch on device ---
        # ReduceScatter over the batch group: rank r keeps the r-th quarter
        # of the flattened [S, DIM] buffer = rows 512r..512(r+1).
        nc.gpsimd.collective_compute(
            "ReduceScatter",
            mybir.AluOpType.add,
            replica_groups=[[0, 1, 2, 3], [4, 5, 6, 7]],
            ins=[out_pre[:].opt()],
            outs=[out_rs[:].opt()],
        )
        nc.sync.dma_start(out=out_d[:], in_=out_rs[:])

    _split_excess_waits(nc)
    return nc


def _split_excess_waits(nc):
    """Compute-engine instructions (Matmult, TensorScalarPtr, ...) only have
    one sync-wait slot in walrus codegen. Split any excess waits onto
    same-engine NoOps inserted just before the instruction."""
    import concourse.mybir as mybir

    n = 0
    for func in nc.m.functions:
        for block in func.blocks:
            out = []
            for inst in block.instructions:
                si = getattr(inst, "sync_info", None)
                if si is not None and si.on_wait and len(si.on_wait) > 1:
                    for w in si.on_wait[:-1]:
                        nop = mybir.InstNoOp(
                            name=f"wsplit_{n}",
                            engine=inst.engine,
                            sync_info=mybir.SyncInfo(on_wait=[w], on_update=[]),
                            bass_nofuse=True,
                        )
                        n += 1
                        out.append(nop)
                    inst.sync_info = mybir.SyncInfo(
                        on_wait=[si.on_wait[-1]], on_update=si.on_update)
                out.append(inst)
            block.instructions[:] = out
    return n


def _prep_inputs(q, k, v, Wq, Wk, Wv, bq, bk, bv, Wo):
    """Project Q/K/V on host (fp32 BLAS) and pack per-core fp16 inputs."""
    f16 = np.float16
    Qp = np.empty((NC, HPC, 128, S), f16)
    Kp = np.empty((NC, HPC, 128, S), f16)
    Vp = np.empty((NC, HPC, 128, S), f16)
    Wop = np.empty((NC, 2, 128, DIM), f16)
    Wo_rows = Wo.reshape(H, D, DIM)
    for c in range(NC):
        b = c // 4
        h0 = (c % 4) * HPC
        for j in range(HPC):
            h = h0 + j
            # QT[j] = (q Wq + bq)^T = Wq^T q^T + bq[:,None]  -> [d, s]
            Qp[c, j] = np.matmul(Wq[h].T, q[b, h].T) + bq[h][:, None]
            Kp[c, j] = np.matmul(Wk[h].T, k[b, h].T) + bk[h][:, None]
            # block-transposed V: [token%128, (token//128, d)]
            Vp[c, j] = (
                (np.matmul(v[b, h], Wv[h]) + bv[h])
                .reshape(NKT, 128, D).transpose(1, 0, 2).reshape(128, S))
        half = Wo_rows[h0 : h0 + 2] if c < 4 else Wo_rows[h0 + 2 : h0 + 4]
        Wop[c] = half
    return [
        {"QT": Qp[c], "KT": Kp[c], "VT": Vp[c], "wo_half": Wop[c]}
        for c in range(NC)
    ]


def kernel(q, k, v, Wq, Wk, Wv, bq, bk, bv, Wo, bo):
    global _BUILT, LAST_RESULTS
    _import_concourse()
    from concourse.bass_utils import run_bass_kernel_spmd

    args = [np.asarray(x, dtype=np.float32)
            for x in (q, k, v, Wq, Wk, Wv, bq, bk, bv, Wo)]
    if _BUILT is None:
        _BUILT = _build()
    in_maps = _prep_inputs(*args)
    res = run_bass_kernel_spmd(_BUILT, in_maps, core_ids=list(range(NC)),
                               trace=TRACE)
    LAST_RESULTS = res
    bo = np.asarray(bo, dtype=np.float32)
    out = np.stack([
        np.concatenate([res.results[c]["out_s"] for c in range(4)],
                       axis=0).astype(np.float32) + bo,
        np.concatenate([res.results[c]["out_s"] for c in range(4, 8)],
                       axis=0).astype(np.float32) + bo,
    ])
    return out


# revision 4
# speedup vs baseline: 18.5799x; 1.5550x over previous
"""MultiHeadAttention TRN2 kernel.

Math (B=2, H=16, S=2048, D=128, F=256, DIM=2048):
  Q = einsum('bhsf,hfd', q, Wq) + bq ; K likewise ; V = einsum('bhse,hed', v, Wv) + bv
  P = softmax(Q K^T / 16) ; o = P V ; out = concat_h(o) @ Wo + bo

The end-to-end metric is the warm kernel() wall time, dominated by
host<->device transfer over the axon tunnel (~20-40 MB/s), not device
compute (~0.5 ms/core). So the kernel minimizes wire bytes:
  - Q/K/V projections run on host in fp32 BLAS (~11 GFLOP, ~0.15 s) and the
    projected tensors ship as int8 with one fp32 scale per head (24 MB
    instead of 160+ MB of raw fp32 q/k/v plus weights). On device they are
    rescaled to fp16 before the matmuls; measured end-to-end rel err ~3e-3
    against the 2e-2 gate.
  - Each core ships only half of its 4 heads' Wo rows in fp16; a 2-rank
    AllGather between batch-pair cores (c, c+4), which need identical rows,
    rebuilds the full set on device (8 MB instead of 16 MB).
  - The attention + output projection partials are summed across each
    batch's 4-core group with an on-device ReduceScatter; the resulting
    [512, 2048] slice is quantized to int8 with a per-row scale on device
    (8 MB down instead of 128 MB of fp32 partials).

Sharding: core c -> batch b=c//4, heads hg=(c%4)*4 .. +4. Each core runs
attention for its 4 heads and the partial Wo product (contraction over its
4*128 rows of Wo). ReduceScatter(add) over [[0..3],[4..7]] leaves core c
with rows 512*(c%4) .. +512 of its batch's output. Host concatenates the
slices, applies the row scales, and adds bo.

Device layout (per core, packed on the host):
  QT  [4,128,2048] int8 (head j, d, s) = (q Wq + bq)^T / lam_q[j]   KT same
  VT  [4,128,2048] int8 (head j, token%128, (token//128, d)) block-transposed
      so VT[j][:, 128*kt:...] is [token, d] for token-chunk kt
  lam [128, 12] f32: per-head dequant scales (q: cols 0-3, k: 4-7, v: 8-11),
      replicated across partitions
  wo_half [2,128,2048] f16 (j, d, n): heads 0-1 of the group on cores 0-3,
      heads 2-3 on cores 4-7
  out_q [512,2048] int8 + osc [128,4] f32 (row r of out_q has scale
      osc[r%128, r//128] / 127)

All matmuls run fp16 (stationary+moving) into fp32 PSUM.
"""

import os
import sys

import numpy as np

B, H, S, D, F = 2, 16, 2048, 128, 256
DIM = H * D
NC = 8
HPC = 4  # heads per core
SC512 = S // 512  # 4
NKT = S // 128  # 16
SOUT = S // 4  # 512 rows returned per core after ReduceScatter

_BUILT = None
TRACE = False
LAST_RESULTS = None


def _import_concourse():
    try:
        import concourse.bass  # noqa: F401
    except ImportError:
        sys.path.insert(0, "/opt/trn_rl_repo")


def _build():
    _import_concourse()
    from contextlib import ExitStack

    import concourse.bass as bass
    import concourse.mybir as mybir
    import concourse.tile as tile

    f32 = mybir.dt.float32
    f16 = mybir.dt.float16
    i8 = mybir.dt.int8
    AF = mybir.ActivationFunctionType

    nc = bass.Bass(target_bir_lowering=False, num_devices=NC)

    qT_d = nc.dram_tensor("QT", [HPC, 128, S], i8, kind="ExternalInput")
    kT_d = nc.dram_tensor("KT", [HPC, 128, S], i8, kind="ExternalInput")
    vT_d = nc.dram_tensor("VT", [HPC, 128, S], i8, kind="ExternalInput")
    lam_d = nc.dram_tensor("lam", [128, 3 * HPC], f32, kind="ExternalInput")
    wo_d = nc.dram_tensor("wo_half", [2, 128, DIM], f16, kind="ExternalInput")
    out_d = nc.dram_tensor("out_q", [SOUT, DIM], i8, kind="ExternalOutput")
    osc_d = nc.dram_tensor("osc", [128, 4], f32, kind="ExternalOutput")

    with ExitStack() as ctx:
        tc = ctx.enter_context(tile.TileContext(nc))
        consts = ctx.enter_context(tc.tile_pool(name="consts", bufs=1))
        raw = ctx.enter_context(tc.tile_pool(name="raw", bufs=2))
        big = ctx.enter_context(tc.tile_pool(name="big", bufs=2))
        otn_pool = ctx.enter_context(tc.tile_pool(name="otn", bufs=4))
        sm = ctx.enter_context(tc.tile_pool(name="sm", bufs=2))
        wop = ctx.enter_context(tc.tile_pool(name="wop", bufs=8))
        ps = ctx.enter_context(tc.tile_pool(name="ps", bufs=1, space="PSUM"))
        dram = ctx.enter_context(tc.tile_pool(name="dram", bufs=1, space="DRAM"))

        wo_in = dram.tile([2, 128, DIM], f16)
        wo_full = dram.tile([HPC, 128, DIM], f16)
        out_pre = dram.tile([S, DIM], f16)
        out_rs = dram.tile([SOUT, DIM], f16)

        # ---- constants -------------------------------------------------
        ones_full = consts.tile([128, 128], f16)
        nc.vector.memset(ones_full[:], 1.0)
        lam_sb = consts.tile([128, 3 * HPC], f32)
        nc.sync.dma_start(out=lam_sb, in_=lam_d[:])

        # wo rows are shared between batch-pair cores (c, c+4): each ships
        # half, a 2-rank AllGather rebuilds the full [4,128,DIM] on device
        nc.scalar.dma_start(out=wo_in[:], in_=wo_d[:])
        nc.gpsimd.collective_compute(
            "AllGather",
            mybir.AluOpType.bypass,
            replica_groups=[[0, 4], [1, 5], [2, 6], [3, 7]],
            ins=[wo_in[:].opt()],
            outs=[wo_full[:].opt()],
        )

        wo_sb = {}
        for dc in range(DIM // 512):
            for j in range(HPC):
                w = wop.tile([128, 512], f16, tag="wo", bufs=16,
                             name=f"wo{dc}_{j}")
                nc.scalar.dma_start(out=w, in_=wo_full[j, :, dc * 512 : (dc + 1) * 512])
                wo_sb[dc, j] = w

        # ---- P3 group emitter (interleaved into head-3 P2 + tail) ------
        store_q = [nc.gpsimd, nc.sync, nc.scalar]
        p3_state = {"n": 0}
        p3_pending = []

        def emit_p3_group(dc, sc, tail):
            csl = slice(sc * 128, (sc + 1) * 128)
            dsl = slice(dc * 512, (dc + 1) * 512)
            pw = ps.tile([128, 512], f32, tag="w", bufs=2, name=f"pw{dc}_{sc}")
            for j in range(HPC):
                nc.tensor.matmul(pw, otn[j][:, csl], wo_sb[dc, j],
                                 start=(j == 0), stop=(j == HPC - 1))
            ow = sm.tile([128, 512], f16, tag="ow", bufs=3, name=f"ow{dc}_{sc}")
            # during interleave keep drains off ACT (the bottleneck engine)
            if tail and p3_state["n"] % 2 == 0:
                nc.scalar.copy(out=ow, in_=pw)
            else:
                nc.vector.tensor_copy(out=ow, in_=pw)
            store_q[p3_state["n"] % 3].dma_start(out=out_pre[csl, dsl], in_=ow)
            p3_state["n"] += 1

        otn = []

        # ---- P1: load head j's int8 tensors, rescale to fp16 -----------
        def emit_head_dmas(j):
            q8 = raw.tile([128, S], i8, tag="q8", name=f"q8_{j}")
            nc.sync.dma_start(out=q8, in_=qT_d[j])
            k8 = raw.tile([128, S], i8, tag="k8", name=f"k8_{j}")
            nc.gpsimd.dma_start(out=k8, in_=kT_d[j])
            v8 = raw.tile([128, S], i8, tag="v8", name=f"v8_{j}")
            (nc.scalar if j == 0 else nc.sync).dma_start(out=v8, in_=vT_d[j])
            return q8, k8, v8

        def convert_head(j, q8, k8, v8):
            QT = big.tile([128, S], f16, tag="QT", name=f"QT{j}")
            nc.vector.tensor_scalar_mul(out=QT, in0=q8,
                                        scalar1=lam_sb[:, j : j + 1])
            KT = big.tile([128, S], f16, tag="KT", name=f"KT{j}")
            nc.vector.tensor_scalar_mul(out=KT, in0=k8,
                                        scalar1=lam_sb[:, HPC + j : HPC + j + 1])
            Vsb = big.tile([128, S], f16, tag="V", name=f"V{j}")
            nc.vector.tensor_scalar_mul(out=Vsb, in0=v8,
                                        scalar1=lam_sb[:, 2 * HPC + j : 2 * HPC + j + 1])
            return QT, KT, Vsb

        hd = emit_head_dmas(0)
        cv = convert_head(0, *hd)
        for j in range(HPC):
            QT, KT, Vsb = cv
            if j + 1 < HPC:
                hd = emit_head_dmas(j + 1)
                cv = convert_head(j + 1, *hd)

            # ---- P2: attention head j ----------------------------------
            oTn = otn_pool.tile([128, S], f16, tag="otn", name=f"oTn{j}")
            otn.append(oTn)
            for qc in range(SC512):
                qsl = slice(qc * 512, (qc + 1) * 512)
                po = ps.tile([128, 512], f32, tag="o", bufs=2, name=f"po{j}_{qc}")
                pr = ps.tile([128, 512], f32, tag="r", bufs=1, name=f"pr{j}_{qc}")

                def emit_pscore(kt):
                    csl = slice(kt * 128, (kt + 1) * 128)
                    t = ps.tile([128, 512], f32, tag="s", bufs=3,
                                name=f"ps{j}_{qc}_{kt}")
                    nc.tensor.matmul(t, KT[:, csl], QT[:, qsl],
                                     start=True, stop=True)
                    return t

                # software pipeline: pscore(kt+1) is emitted before po(kt)
                # so PE's in-order queue keeps ACT fed with score tiles
                # while po waits on exp(kt); otherwise every exp gets a
                # PE->ACT round-trip bubble on the bottleneck engine
                cur = emit_pscore(0)
                for kt in range(NKT):
                    csl = slice(kt * 128, (kt + 1) * 128)
                    pT = sm.tile([128, 512], f16, tag="pT", bufs=3, name=f"pT{j}_{qc}_{kt}")
                    nc.scalar.activation(out=pT, in_=cur, func=AF.Exp,
                                         bias=0.0, scale=0.0625)
                    if kt + 1 < NKT:
                        cur = emit_pscore(kt + 1)
                    nc.tensor.matmul(po, Vsb[:, csl], pT,
                                     start=(kt == 0), stop=(kt == NKT - 1))
                    nc.tensor.matmul(pr, ones_full, pT,
                                     start=(kt == 0), stop=(kt == NKT - 1))
                    # PE slack under the ACT exp bottleneck: fold one output
                    # projection group per kt slot once its tokens are done
                    if p3_pending:
                        emit_p3_group(*p3_pending.pop(0), tail=False)
                rr = sm.tile([128, 512], f32, tag="rr_sb", bufs=2, name=f"rr{j}_{qc}")
                nc.vector.reciprocal(out=rr, in_=pr)
                nc.vector.tensor_mul(out=oTn[:, qsl], in0=po, in1=rr)
                if j == HPC - 1:
                    p3_pending.extend(
                        (dc, sc)
                        for sc in range(qc * 4, (qc + 1) * 4)
                        for dc in range(DIM // 512))

        # ---- P3 tail: groups not hidden inside P2 ----------------------
        while p3_pending:
            emit_p3_group(*p3_pending.pop(0), tail=True)

        # ---- P4: sum the 4 per-core partials of this batch on device ---
        # ReduceScatter over the batch group: rank r keeps the r-th quarter
        # of the flattened [S, DIM] buffer = rows 512r..512(r+1).
        nc.gpsimd.collective_compute(
            "ReduceScatter",
            mybir.AluOpType.add,
            replica_groups=[[0, 1, 2, 3], [4, 5, 6, 7]],
            ins=[out_pre[:].opt()],
            outs=[out_rs[:].opt()],
        )

        # ---- P5: int8-quantize the result slice with per-row scales ----
        osc_sb = consts.tile([128, 4], f32)
        for t in range(SOUT // 128):
            ot = sm.tile([128, DIM], f16, tag="oq_in", bufs=2, name=f"ot{t}")
            nc.sync.dma_start(out=ot, in_=out_rs[t * 128 : (t + 1) * 128, :])
            am = sm.tile([128, 1], f32, tag="oq_am", bufs=2, name=f"am{t}")
            nc.vector.tensor_reduce(out=am, in_=ot, axis=mybir.AxisListType.X,
                                    op=mybir.AluOpType.max,
                                    apply_absolute_value=True)
            inv = sm.tile([128, 1], f32, tag="oq_inv", bufs=2, name=f"inv{t}")
            nc.vector.reciprocal(out=inv, in_=am)
            scl = sm.tile([128, 1], f32, tag="oq_scl", bufs=2, name=f"scl{t}")
            nc.vector.tensor_scalar_mul(out=scl, in0=inv, scalar1=127.0)
            oq = sm.tile([128, DIM], i8, tag="oq_out", bufs=2, name=f"oq{t}")
            nc.vector.tensor_scalar_mul(out=oq, in0=ot, scalar1=scl[:, 0:1])
            nc.scalar.dma_start(out=out_d[t * 128 : (t + 1) * 128, :], in_=oq)
            nc.vector.tensor_copy(out=osc_sb[:, t : t + 1], in_=am)
        nc.sync.dma_start(out=osc_d[:], in_=osc_sb)

    _split_excess_waits(nc)
    return nc


def _split_excess_waits(nc):
    """Compute-engine instructions (Matmult, TensorScalarPtr, ...) only have
    one sync-wait slot in walrus codegen. Split any excess waits onto
    same-engine NoOps inserted just before the instruction."""
    import concourse.mybir as mybir

    n = 0
    for func in nc.m.functions:
        for block in func.blocks:
            out = []
            for inst in block.instructions:
                si = getattr(inst, "sync_info", None)
                if si is not None and si.on_wait and len(si.on_wait) > 1:
                    for w in si.on_wait[:-1]:
                        nop = mybir.InstNoOp(
                            name=f"wsplit_{n}",
                            engine=inst.engine,
                            sync_info=mybir.SyncInfo(on_wait=[w], on_update=[]),
                            bass_nofuse=True,
                        )
                        n += 1
                        out.append(nop)
                    inst.sync_info = mybir.SyncInfo(
                        on_wait=[si.on_wait[-1]], on_update=si.on_update)
                out.append(inst)
            block.instructions[:] = out
    return n


def _quant_head(dst, lam_col, x):
    """int8-quantize one head's [128, S] fp32 tensor with a single scale."""
    a = np.abs(x).max()
    lam = a / 127.0 if a > 0 else 1.0
    np.rint(x * (1.0 / lam), out=x)
    dst[...] = x.astype(np.int8)
    return lam


def _prep_inputs(q, k, v, Wq, Wk, Wv, bq, bk, bv, Wo):
    """Project Q/K/V on host (fp32 BLAS), int8-quantize per head, and pack
    per-core inputs."""
    Qp = np.empty((NC, HPC, 128, S), np.int8)
    Kp = np.empty((NC, HPC, 128, S), np.int8)
    Vp = np.empty((NC, HPC, 128, S), np.int8)
    Lam = np.empty((NC, 128, 3 * HPC), np.float32)
    Wop = np.empty((NC, 2, 128, DIM), np.float16)
    Wo_rows = Wo.reshape(H, D, DIM)
    for c in range(NC):
        b = c // 4
        h0 = (c % 4) * HPC
        for j in range(HPC):
            h = h0 + j
            # QT[j] = (q Wq + bq)^T = Wq^T q^T + bq[:,None]  -> [d, s]
            Lam[c, :, j] = _quant_head(
                Qp[c, j], j, np.matmul(Wq[h].T, q[b, h].T) + bq[h][:, None])
            Lam[c, :, HPC + j] = _quant_head(
                Kp[c, j], j, np.matmul(Wk[h].T, k[b, h].T) + bk[h][:, None])
            # block-transposed V: [token%128, (token//128, d)]
            Lam[c, :, 2 * HPC + j] = _quant_head(
                Vp[c, j], j,
                (np.matmul(v[b, h], Wv[h]) + bv[h])
                .reshape(NKT, 128, D).transpose(1, 0, 2).reshape(128, S))
        half = Wo_rows[h0 : h0 + 2] if c < 4 else Wo_rows[h0 + 2 : h0 + 4]
        Wop[c] = half
    return [
        {"QT": Qp[c], "KT": Kp[c], "VT": Vp[c], "lam": Lam[c],
         "wo_half": Wop[c]}
        for c in range(NC)
    ]


def kernel(q, k, v, Wq, Wk, Wv, bq, bk, bv, Wo, bo):
    global _BUILT, LAST_RESULTS
    _import_concourse()
    from concourse.bass_utils import run_bass_kernel_spmd

    args = [np.asarray(x, dtype=np.float32)
            for x in (q, k, v, Wq, Wk, Wv, bq, bk, bv, Wo)]
    if _BUILT is None:
        _BUILT = _build()
    in_maps = _prep_inputs(*args)
    res = run_bass_kernel_spmd(_BUILT, in_maps, core_ids=list(range(NC)),
                               trace=TRACE)
    LAST_RESULTS = res
    bo = np.asarray(bo, dtype=np.float32)

    def assemble(cores):
        parts = []
        for c in cores:
            oq = res.results[c]["out_q"].astype(np.float32)
            # row r of out_q has scale osc[r%128, r//128] / 127
            scales = res.results[c]["osc"].T.reshape(SOUT, 1) / 127.0
            parts.append(oq * scales)
        return np.concatenate(parts, axis=0) + bo

    return np.stack([assemble(range(4)), assemble(range(4, 8))])
